# revision 15
# baseline (speedup 1.0000x reference)
"""Trainium2 Bass kernel for nn_DWTExtractor: 2-level Haar DWT + bilinear 2x upsample.

Input  x: (32, 1, 1024, 1024) fp32
Output y: (32, 6, 512, 512) fp32 = [cH1, cV1, cD1, cH2u, cV2u, cD2u]

Sharding: pure batch data-parallel, 4 images per core across 8 cores.

v4 design (all band combines folded into PSUM accumulation):
  - fp16 datapath (host converts, ~1e-3 rel err).
  - L1 Haar per 128-row block: U = [cA1|cV1] and V = [cH1|cD1] are each
    built by TWO accumulating matmuls (fused sum/diff weight WF on the
    even cols, +-WF on the odd cols). No vector combines at all; PSUM
    evacuation is a pure f32->f16 copy, alternating ACT (U) / DVE (V)
    so the copy rate never stalls the PE stream (PE p-state ramps only
    when continuously busy: 1.2 -> 2.4 GHz after ~3us).
  - L2 same trick on cA1 (Ustg parts 0..63, zero-padded weights): per-g
    psum HD = [cH2 | cD2-on-wrong-half], psum V = [junk cA2 | cV2].
    cH2 copies lane-aligned straight into row-major b3all; cV2/cD2 copy
    to VDtmp and take 4 small shift DMAs. cA2 never materialized.
  - W-upsample: t3 = 3*b3 (tensor_scalar, 4x mode) + two shifted adds
    (tensor_tensor, 2x mode) into parity-BLOCKED wall [128,(s)(h)(e|o)];
    scalar_tensor_tensor measured 1x-only - avoid it.
  - H-upsample: 12 matmuls + halo row swap; evacuation APs do the e/o
    interleave for free.
  - Hazard notes: PSUM accumulation groups that interleave must sit in
    different BANKS (same-bank interleave corrupts); DMA dst APs need a
    single uniform partition shift; >3-dim DMA APs don't balance.
  - Triggers: Sync = input ONLY (never blocks input streaming), GPSIMD =
    band outputs + VD shifts + halo (SWDGE), ACT/DVE = pure compute.
"""

import numpy as np

import concourse.bass as bass
import concourse.tile as tile
import concourse.mybir as mybir
from concourse import bacc, bass_utils

F32 = mybir.dt.float32
F16 = mybir.dt.float16
AL = mybir.AluOpType

B, H, W = 32, 1024, 1024
NCORES = 8
IMG = B // NCORES  # images per core
HL, WL = H // 2, W // 2  # 512 (level-1 band size)
H2, W2 = H // 4, W // 4  # 256 (level-2 band size)
P = 128


def _build_w16() -> np.ndarray:
    """(128, 14*128) fp16:
    WF | WFN | A0 B0 An0 Bn0 | A1 B1 An1 Bn1 | U0 U1p U2p U3.

    WF: out parts [row-pair sums | row-pair diffs].  A/B (parity q):
    S2-pairs at out parts 64q+i, D2-pairs at 64(1-q)+i; An/Bn negated.
    """
    wf = np.zeros((P, P), np.float16)
    for i in range(64):
        wf[2 * i, i] = 0.5
        wf[2 * i + 1, i] = 0.5
        wf[2 * i, 64 + i] = 0.5
        wf[2 * i + 1, 64 + i] = -0.5

    wl2 = []
    for q in (0, 1):
        a = np.zeros((P, P), np.float16)
        bq = np.zeros((P, P), np.float16)
        so, do = 64 * q, 64 * (1 - q)
        for i in range(32):
            a[2 * i, so + i] = 0.5
            a[2 * i + 1, so + i] = 0.5
            a[2 * i, do + i] = 0.5
            a[2 * i + 1, do + i] = -0.5
            bq[2 * i, so + 32 + i] = 0.5
            bq[2 * i + 1, so + 32 + i] = 0.5
            bq[2 * i, do + 32 + i] = 0.5
            bq[2 * i + 1, do + 32 + i] = -0.5
        wl2 += [a, bq, -a, -bq]

    u_full = np.zeros((H2, HL), np.float32)
    for m in range(HL):
        k = m // 2
        taps = [(k, 0.75), (k - 1, 0.25)] if m % 2 == 0 else [(k, 0.75), (k + 1, 0.25)]
        for src, wgt in taps:
            u_full[min(max(src, 0), H2 - 1), m] += wgt
    u_full *= 0.25
    u0 = u_full[0:128, 0:128].astype(np.float16)
    u1p = u_full[0:128, 128:256].astype(np.float16)
    u1p[0, :] = u_full[128, 128:256].astype(np.float16)  # halo tap row
    u2p = u_full[128:256, 256:384].astype(np.float16)
    u2p[127, :] = u_full[127, 256:384].astype(np.float16)  # halo tap row
    u3 = u_full[128:256, 384:512].astype(np.float16)

    return np.concatenate([wf, -wf] + wl2 + [u0, u1p, u2p, u3], axis=1)


def build_nc() -> "bacc.Bacc":
    nc = bacc.Bacc(
        "TRN2", target_bir_lowering=False, debug=False, num_devices=NCORES,
        name="dwt_extractor",
    )
    x_d = nc.dram_tensor("xc", [IMG, H, W], F16, kind="ExternalInput")
    w16_d = nc.dram_tensor("w16", [P, 14 * P], F16, kind="ExternalInput")
    y_d = nc.dram_tensor("yc", [IMG, 6, HL, WL], F16, kind="ExternalOutput")

    with tile.TileContext(nc) as tc:
        with (
            tc.tile_pool(name="consts", bufs=1) as cpool,
            tc.tile_pool(name="xin", bufs=3) as xpool,
            tc.tile_pool(name="uv", bufs=2) as uvpool,
            tc.tile_pool(name="vdt", bufs=2) as vdpool,
            tc.tile_pool(name="b3", bufs=2) as b3pool,
            tc.tile_pool(name="t3p", bufs=2) as t3pool,
            tc.tile_pool(name="wtile", bufs=2) as wpool,
            tc.tile_pool(name="stg2", bufs=2) as stpool,
            tc.tile_pool(name="psL1", bufs=4, space="PSUM") as psL1,
            tc.tile_pool(name="psL2", bufs=2, space="PSUM") as psL2,
            tc.tile_pool(name="psUp", bufs=2, space="PSUM") as psUp,
        ):
            w16 = cpool.tile([P, 14 * P], F16)
            nc.sync.dma_start(w16[:], w16_d[:])
            blk = lambda i: w16[:, i * P : (i + 1) * P]
            WF, WFN = blk(0), blk(1)
            WL2 = [(blk(2), blk(3), blk(4), blk(5)),
                   (blk(6), blk(7), blk(8), blk(9))]  # [q] -> (A, B, An, Bn)
            U0, U1p, U2p, U3 = blk(10), blk(11), blk(12), blk(13)

            def l1_half(b, hf, Ustg, Vstg):
                """Four 128-row blocks: one load; per block U/V built by
                accumulating matmuls, evac copy ACT (U) / DVE (V)."""
                xu = xpool.tile([P, 4096], F16, tag="x")
                src = x_d[b, 512 * hf : 512 * (hf + 1), :]
                nc.sync.dma_start(
                    xu[:].rearrange("p (t w) -> p t w", t=4),
                    src.rearrange("(t p) w -> p t w", t=4),
                )
                for t in range(4):
                    u = 4 * hf + t
                    xb = xu[:, 1024 * t : 1024 * (t + 1)]
                    xe, xo = xb[:, 0:1024:2], xb[:, 1:1024:2]
                    psU = psL1.tile([P, 512], F32, tag="ps")
                    psV = psL1.tile([P, 512], F32, tag="ps")
                    nc.tensor.matmul(psU[:], WF, xe, start=True, stop=False)
                    nc.tensor.matmul(psV[:], WF, xe, start=True, stop=False)
                    nc.tensor.matmul(psU[:], WF, xo, start=False, stop=True)
                    nc.tensor.matmul(psV[:], WFN, xo, start=False, stop=True)
                    o = 512 * u
                    nc.scalar.copy(Ustg[:, o : o + 512], psU[:])
                    nc.vector.tensor_copy(Vstg[:, o : o + 512], psV[:])

            def l2_group(g, Ustg, b3all, VDtmp):
                """cA1 rows 128g..+127 -> psum HD = [cH2 | cD2'], psum V =
                [cA2junk | cV2']; copies go lane-aligned / to VDtmp."""
                q, s = g % 2, g // 2
                WA, WB, WAn, WBn = WL2[q]
                ue0 = Ustg[:, 1024 * g : 1024 * g + 512]
                ue1 = Ustg[:, 1024 * g + 512 : 1024 * g + 1024]
                e0, o0 = ue0[:, 0:512:2], ue0[:, 1:512:2]
                e1, o1 = ue1[:, 0:512:2], ue1[:, 1:512:2]
                psHD = psL2.tile([P, 256], F32, tag="ps2", padded_shape=[P, 512])
                psV = psL2.tile([P, 256], F32, tag="ps2", padded_shape=[P, 512])
                # grouped by weight; HD and V groups sit in different banks
                nc.tensor.matmul(psHD[:], WA, e0, start=True, stop=False)
                nc.tensor.matmul(psV[:], WA, e0, start=True, stop=False)
                nc.tensor.matmul(psV[:], WA, o0, start=False, stop=False)
                nc.tensor.matmul(psHD[:], WB, e1, start=False, stop=False)
                nc.tensor.matmul(psV[:], WB, e1, start=False, stop=False)
                nc.tensor.matmul(psV[:], WB, o1, start=False, stop=True)
                nc.tensor.matmul(psHD[:], WAn, o0, start=False, stop=False)
                nc.tensor.matmul(psHD[:], WBn, o1, start=False, stop=True)
                so, do = 64 * q, 64 * (1 - q)
                # cH2 lane-aligned into b3all H block
                nc.scalar.copy(
                    b3all[so : so + 64, 768 * s : 768 * s + 256],
                    psHD[so : so + 64, :])
                # cD2 / cV2 on the wrong half -> VDtmp (shifted later)
                nc.vector.tensor_copy(
                    VDtmp[do : do + 64, 1024 + 512 * s + 256 * q :
                          1024 + 512 * s + 256 * q + 256],
                    psHD[do : do + 64, :])
                nc.vector.tensor_copy(
                    VDtmp[do : do + 64, 512 * s + 256 * q :
                          512 * s + 256 * q + 256],
                    psV[do : do + 64, :])

            def wup_half(s, b3all, VDtmp, wall, t3):
                """Shift cV2/cD2 of w-tile s into b3all, W-upsample that
                half into wall cols [1536s : 1536s+1536]."""
                for q in (0, 1):
                    src = VDtmp[64 * (1 - q) : 64 * (2 - q), :].rearrange(
                        "p (bb sc qc c) -> p bb sc qc c",
                        bb=2, sc=2, qc=2)[:, :, s, q, :]
                    dst = b3all[64 * q : 64 * q + 64, :].rearrange(
                        "p (sb h c) -> p sb h c", sb=2, h=3)[:, s, 1:3, :]
                    nc.sync.dma_start(dst, src)
                s4 = b3all[:, 768 * s : 768 * s + 768].rearrange(
                    "p (h c) -> p h c", h=3)
                t4 = t3[:, 768 * s : 768 * s + 768].rearrange(
                    "p (h c) -> p h c", h=3)
                d4 = wall[:, 1536 * s : 1536 * s + 1536].rearrange(
                    "p (h x) -> p h x", h=3)
                nc.vector.tensor_scalar_mul(t4[:], s4[:], 3.0)
                # even block: wu[2c] = 3b[c] + b[c-1]; odd: wu[2c+1] = 3b[c] + b[c+1]
                nc.vector.tensor_tensor(
                    d4[:, :, 1:256], t4[:, :, 1:256], s4[:, :, 0:255], AL.add)
                nc.vector.tensor_tensor(
                    d4[:, :, 256:511], t4[:, :, 0:255], s4[:, :, 1:256], AL.add)
                nc.vector.tensor_scalar_mul(
                    d4[:, :, 0:512:511], s4[:, :, 0:256:255], 4.0)

            def evac_up(st, j, src_ap, k):
                # interleave even|odd parity blocks while evacuating
                dst = st[:, 512 * j : 512 * j + 512].rearrange(
                    "p (c par) -> p par c", par=2)
                src = src_ap.rearrange("p (par c) -> p par c", par=2)
                if k % 3 == 2:
                    nc.vector.tensor_copy(dst, src)
                else:
                    nc.scalar.copy(dst, src)

            def stage_b1(b, wall, sts):
                """H-up blocks 0 and 3 + halo row swaps for image b."""
                k = 0
                for j, Uw, wo in ((0, U0, 0), (3, U3, 1536)):
                    for band in range(3):
                        if j == 0:
                            st = stpool.tile([P, 2048], F16,
                                             tag=f"s2b{band}", name=f"s2b{band}")
                            sts.append(st)
                        else:
                            st = sts[band]
                        up = psUp.tile([P, 512], F32, tag="up")
                        nc.tensor.matmul(
                            up[:], Uw, wall[:, wo + 512 * band : wo + 512 * (band + 1)],
                            start=True, stop=True)
                        evac_up(st, j, up[:], k)
                        k += 1
                # halo: w0 row0 <- w1 row0; w1 row127 <- w0 row127
                nc.gpsimd.dma_start(wall[0:1, 0:1536], wall[0:1, 1536:3072])
                nc.gpsimd.dma_start(wall[127:128, 1536:3072], wall[127:128, 0:1536])

            def stage_b2(b, wall, sts):
                """H-up blocks 1 and 2 (halo'd) + output DMA for image b."""
                k = 3
                for j, Uw, wo in ((1, U1p, 0), (2, U2p, 1536)):
                    for band in range(3):
                        up = psUp.tile([P, 512], F32, tag="up")
                        nc.tensor.matmul(
                            up[:], Uw, wall[:, wo + 512 * band : wo + 512 * (band + 1)],
                            start=True, stop=True)
                        evac_up(sts[band], j, up[:], k)
                        k += 1
                for band in range(3):
                    dst = y_d[b, 3 + band]
                    nc.gpsimd.dma_start(
                        dst.rearrange("(u p) w -> p u w", u=4),
                        sts[band][:].rearrange("p (u w) -> p u w", u=4))

            pending = None
            for b in range(IMG):
                Ustg = uvpool.tile([P, 4096], F16, tag="U", name="Ustg")
                Vstg = uvpool.tile([P, 4096], F16, tag="V", name="Vstg")
                b3all = b3pool.tile([P, 1536], F16, tag="b3", name="b3all")
                VDtmp = vdpool.tile([P, 2048], F16, tag="vd", name="VDtmp")
                wall = wpool.tile([P, 3072], F16, tag="wall", name="wall")
                t3 = t3pool.tile([P, 1536], F16, tag="t3", name="t3")
                l1_half(b, 0, Ustg, Vstg)
                l2_group(0, Ustg, b3all, VDtmp)
                l2_group(1, Ustg, b3all, VDtmp)
                wup_half(0, b3all, VDtmp, wall, t3)
                if pending is not None:
                    stage_b1(pending[0], pending[1], pending[2])
                l1_half(b, 1, Ustg, Vstg)
                l2_group(2, Ustg, b3all, VDtmp)
                l2_group(3, Ustg, b3all, VDtmp)
                wup_half(1, b3all, VDtmp, wall, t3)
                # L1 band outputs: cH1=V[0:64], cV1=U[64:128], cD1=V[64:128]
                for band, (stg, lo) in enumerate(
                        ((Vstg, 0), (Ustg, 64), (Vstg, 64))):
                    src = stg[lo : lo + 64, :].rearrange(
                        "p (u w) -> p u w", u=8)
                    nc.gpsimd.dma_start(
                        y_d[b, band].rearrange("(u p) w -> p u w", u=8), src)
                if pending is not None:
                    stage_b2(pending[0], pending[1], pending[2])
                pending = (b, wall, [])
            stage_b1(pending[0], pending[1], pending[2])
            stage_b2(pending[0], pending[1], pending[2])

    nc.compile()
    return nc


_NC_CACHE = None
LAST_RESULTS = None


def kernel(**inputs) -> np.ndarray:
    global _NC_CACHE, LAST_RESULTS
    trace = bool(inputs.pop("_trace", False))
    x = np.asarray(inputs["x"])
    assert x.shape == (B, 1, H, W), x.shape
    x16 = np.ascontiguousarray(x.astype(np.float16))
    if _NC_CACHE is None:
        _NC_CACHE = build_nc()
    nc = _NC_CACHE
    w16 = _build_w16()
    in_maps = [
        {"xc": np.ascontiguousarray(x16[IMG * c : IMG * (c + 1), 0]), "w16": w16}
        for c in range(NCORES)
    ]
    res = bass_utils.run_bass_kernel_spmd(
        nc, in_maps, core_ids=list(range(NCORES)), trace=trace
    )
    LAST_RESULTS = res
    out = np.concatenate([res.results[c]["yc"] for c in range(NCORES)], axis=0)
    return out.astype(np.float32)


if __name__ == "__main__":
    rng = np.random.default_rng(0)
    x = rng.standard_normal((B, 1, H, W), dtype=np.float32)
    y = kernel(x=x)
    print("kernel output:", y.shape, y.dtype)
